# revision 1
# baseline (speedup 1.0000x reference)
"""Fused single-head attention kernel for 8 TRN2 NeuronCores.

Problem: B=4, S=2048, D=1024 attention:
    Q = x @ Wq.T + bq; K = x @ Wk.T + bk; V = x @ Wv.T + bv
    out = softmax(Q K^T / sqrt(D)) @ V

Sharding (no cross-core traffic): core c handles batch b = c//2 and
query half h = c%2 (1024 queries).

The kernel uses an algebraic refactoring that removes the K and V
projections (and with them any need to exchange K/V between the two
cores of a batch pair):

  logits = (x_q Wq^T + bq)(x_k Wk^T + bk)^T / sqrt(D)
         = x_q M2 x_k^T + x_k.z + (per-q terms), M2 = Wq^T Wk / sqrt(D)
  - the per-QUERY additive terms (x_q Wq^T bk and bq.bk) are constant
    along the softmax axis and drop out of the softmax entirely;
  - the per-KEY term x_k.(Wk^T bq)/sqrt(D) = x_k.z survives and is
    folded into Y (Y' = M2^T x_q^T + z 1^T, added as the per-partition
    bias of Y's evacuation), so it costs nothing.
  M2 [d,d] and z [d] depend only on the weights and are precomputed on
  the host (f64) - weight fusion, no runtime data involved.

  out = softmax @ (x Wv^T) + bv = ((P x) Wv^T)/rowsum(P) + bv
  so V is never materialized: first tmp = P^T.T @ x (attention-weighted
  inputs), then one [1024,1024] projection by Wv^T at the end.

Per-core device FLOPs drop from 15.0 to 12.9 GFLOP and all tensors are
core-local. x is passed in two host-prepared layouts, xT [d,s] and
xN [s,d], both rotated along s by -h*QH so this core's queries are
always positions 0:QH (a uniform slice; attention is permutation-
invariant along the key axis, and xT/xN agree on the rotation).

Device dataflow (all matmul inputs bf16, accumulation fp32):
  Y[dc,q] = M2^T.T @ xqT + z      (128 MMs, bias in the evac)
  per q-block of 512 (pass1/2/3):
    pass1: attT[k,q] += xT_slice^T.T @ Y  (k on partitions -> softmax
      sums and the downstream matmuls all need no transposes);
      PT = exp(attT) (ScalarE, PSUM->SBUF, kept for pass 2);
      rowsum via ones-stationary matmul -> S_row [1,512] (PSUM, 1 bank);
      1/S: evac, transpose 128-slices via K=1 matmul, DVE reciprocal
    pass2: tmpT[d,q] += xN_slice^T.T @ PT  (PSUM over 16 k-tiles)
    pass3: out[q,e] += tmpT_slice^T.T @ WvT; evac with
      out = out * (1/S) + bv -> DMA to DRAM
"""

import os
import sys

for _p in ("/opt/trn_rl_repo", "/root/.axon_site/_ro/trn_rl_repo"):
    if os.path.isdir(_p) and _p not in sys.path:
        sys.path.insert(0, _p)

import numpy as np
import ml_dtypes

import concourse.bass as bass
import concourse.tile as tile
from concourse import bacc, mybir
from concourse.bass_utils import run_bass_kernel_spmd

BF16 = ml_dtypes.bfloat16
F32 = mybir.dt.float32
CDT = mybir.dt.bfloat16

B, S, D = 4, 2048, 1024
N_CORES = 8
P = 128
DT = D // P          # 8 d-tiles (contraction)
KT_N = S // P        # 16 k-tiles
QH = S // 2          # 1024 queries per core
QB = 512             # q-block for phase B
NQB = QH // QB       # 2 q-blocks
QS = QB // P         # 4 q-subtiles per block

_NC_CACHE = {}


def build_nc(reps: int = 1, mode: str = "full"):
    nc = bacc.Bacc("TRN2", target_bir_lowering=False, debug=False,
                   num_devices=N_CORES)
    Exp = mybir.ActivationFunctionType.Exp
    Copy = mybir.ActivationFunctionType.Copy

    xT_d = nc.dram_tensor("xT", [D, S], CDT, kind="ExternalInput").ap()
    xN_d = nc.dram_tensor("xN", [S, D], CDT, kind="ExternalInput").ap()
    m2_d = nc.dram_tensor("M2", [D, D], CDT, kind="ExternalInput").ap()
    wvT_d = nc.dram_tensor("WvT", [D, D], CDT, kind="ExternalInput").ap()
    z_d = nc.dram_tensor("z2", [P, DT], F32, kind="ExternalInput").ap()
    bv_d = nc.dram_tensor("bvb", [P, D], F32, kind="ExternalInput").ap()
    out_d = nc.dram_tensor("out", [QH, D], F32, kind="ExternalOutput").ap()

    with tile.TileContext(nc) as tc:
        with (
            tc.tile_pool(name="resident", bufs=1) as res,
            tc.tile_pool(name="wpool", bufs=2) as wpool,
            tc.tile_pool(name="pt", bufs=2) as ptpool,
            tc.tile_pool(name="tm", bufs=2) as tmpool,
            tc.tile_pool(name="osb", bufs=4) as opool,
            tc.tile_pool(name="small", bufs=4) as spool,
            tc.tile_pool(name="ps", bufs=2, space="PSUM") as psA,
            tc.tile_pool(name="ptm", bufs=4, space="PSUM") as psT,
            tc.tile_pool(name="pss", bufs=2, space="PSUM") as psS,
        ):
            # ---- resident loads (once) ----
            # order matters for the single-shot prologue: the Y matmuls
            # need z + M2 + xqT first; xT (pass1) next; xN (pass2) and
            # bv (epilogue) last. xT/xN ride the scalar HWDGE queue so
            # they stream in parallel with the sync-queue loads.
            z_sb = res.tile([P, DT], F32, tag="z", name="z_sb")
            nc.scalar.dma_start(z_sb[:], z_d[:, :])
            m2 = [wpool.tile([P, D], CDT, tag=f"w{d}", name=f"m2_{d}")
                  for d in range(DT)]
            for d in range(DT):
                nc.sync.dma_start(m2[d][:], m2_d[d * P:(d + 1) * P, :])
            # xt halves: Y's rhs only needs columns 0:QH, so land those
            # first (2KB/partition descriptors either way)
            xt = [res.tile([P, S], CDT, tag=f"xt{d}", name=f"xt{d}")
                  for d in range(DT)]
            for half in range(2):
                cols = slice(half * QH, (half + 1) * QH)
                for d in range(DT):
                    (nc.sync if d % 2 else nc.scalar).dma_start(
                        xt[d][:, cols], xT_d[d * P:(d + 1) * P, cols])
            xn = [res.tile([P, D], CDT, tag=f"xn{k}", name=f"xn{k}")
                  for k in range(KT_N)]
            for k in range(KT_N):
                (nc.sync if k % 2 else nc.scalar).dma_start(
                    xn[k][:], xN_d[k * P:(k + 1) * P, :])
            bv_sb = res.tile([P, D], F32, tag="bv", name="bv_sb")
            nc.scalar.dma_start(bv_sb[:], bv_d[:, :])
            ones = res.tile([P, 1], CDT, tag="ones", name="ones")
            nc.vector.memset(ones[:], 1.0)
            one11 = res.tile([1, 1], F32, tag="one11", name="one11")
            nc.vector.memset(one11[:], 1.0)

            yt = [res.tile([P, QH], CDT, tag=f"yt{d}", name=f"yt{d}")
                  for d in range(DT)]

            a_iters = range(reps) if mode in ("full", "A") else range(1)
            b_iters = range(reps) if mode in ("full", "B") else range(1)
            wv = None
            for _i_rep, _rep in enumerate(a_iters):
                # ---- Y = M2^T.T @ xqT : [dc, q] ----
                if _i_rep > 0:
                    m2 = [wpool.tile([P, D], CDT, tag=f"w{d}",
                                     name=f"m2_{d}") for d in range(DT)]
                    for d in range(DT):
                        nc.sync.dma_start(m2[d][:],
                                          m2_d[d * P:(d + 1) * P, :])
                for dc in range(DT):
                    for sb in range(QH // 512):
                        ps = psA.tile([P, 512], F32, tag="ps", name="ps")
                        for d in range(DT):
                            nc.tensor.matmul(
                                ps[:],
                                lhsT=m2[d][:, dc * P:(dc + 1) * P],
                                rhs=xt[d][:, sb * 512:(sb + 1) * 512],
                                start=(d == 0), stop=(d == DT - 1))
                        nc.scalar.activation(
                            yt[dc][:, sb * 512:(sb + 1) * 512], ps[:],
                            mybir.ActivationFunctionType.Identity,
                            bias=z_sb[:, dc:dc + 1])
                # WvT loads reuse the w{d} slots once M2 is consumed
                wv = [wpool.tile([P, D], CDT, tag=f"w{d}", name=f"wv_{d}")
                      for d in range(DT)]
                for d in range(DT):
                    nc.sync.dma_start(wv[d][:], wvT_d[d * P:(d + 1) * P, :])

            for _rep in b_iters:
                for qb in range(NQB):
                    # ---- pass 1: scores, exp(+v3 bias), row sums ----
                    srow_ps = psS.tile([1, QB], F32, tag="pss", name="srow_ps")
                    pts = []
                    for k in range(KT_N):
                        psa = psA.tile([P, QB], F32, tag="ps", name="psa")
                        for d in range(DT):
                            nc.tensor.matmul(
                                psa[:],
                                lhsT=xt[d][:, k * P:(k + 1) * P],
                                rhs=yt[d][:, qb * QB:(qb + 1) * QB],
                                start=(d == 0), stop=(d == DT - 1))
                        pt_sb = ptpool.tile([P, QB], CDT, tag=f"pt{k}",
                                            name=f"pt_sb{k}")
                        nc.scalar.activation(pt_sb[:], psa[:], Exp)
                        pts.append(pt_sb)
                    # rowsum as one batched chain (keeps pass1's PE
                    # stream pure attT; exp(k)->srow dep leaves pass1)
                    for k in range(KT_N):
                        nc.tensor.matmul(
                            srow_ps[:], lhsT=ones[:], rhs=pts[k][:],
                            start=(k == 0), stop=(k == KT_N - 1))
                    srow_sb = spool.tile([1, QB], F32, tag="srow",
                                         name="srow_sb")
                    nc.scalar.copy(srow_sb[:], srow_ps[:])
                    recs = []
                    for qs in range(QS):
                        scol_ps = psA.tile([P, 1], F32, tag="ps",
                                           name="scol_ps")
                        nc.tensor.matmul(
                            scol_ps[:],
                            lhsT=srow_sb[0:1, qs * P:(qs + 1) * P],
                            rhs=one11[:], start=True, stop=True)
                        rec = spool.tile([P, 1], F32, tag="rec", name="rec")
                        nc.vector.reciprocal(rec[:], scol_ps[:])
                        recs.append(rec)
                    # ---- pass 2: tmpT[d, q] = sum_k x_k^T P^T ----
                    tms = []
                    for dt_i in range(DT):
                        pst = psT.tile([P, 512], F32, tag="ptm", name="pst")
                        for k in range(KT_N):
                            nc.tensor.matmul(
                                pst[:],
                                lhsT=xn[k][:, dt_i * P:(dt_i + 1) * P],
                                rhs=pts[k][:],
                                start=(k == 0), stop=(k == KT_N - 1))
                        tm = tmpool.tile([P, 512], CDT, tag=f"tm{dt_i}",
                                         name=f"tm{dt_i}")
                        nc.vector.tensor_copy(out=tm[:], in_=pst[:])
                        tms.append(tm)
                    # ---- pass 3: out[q, e] = tmpT^T @ WvT, scaled + bv ----
                    for qs in range(QS):
                        for eb in range(2):
                            pso = psA.tile([P, 512], F32, tag="ps",
                                           name="pso")
                            for dt_i in range(DT):
                                nc.tensor.matmul(
                                    pso[:],
                                    lhsT=tms[dt_i][:, qs * P:(qs + 1) * P],
                                    rhs=wv[dt_i][:, eb * 512:(eb + 1) * 512],
                                    start=(dt_i == 0), stop=(dt_i == DT - 1))
                            osb = opool.tile([P, 512], F32, tag="osb",
                                             name="osb")
                            nc.scalar.activation(osb[:], pso[:], Copy,
                                                 scale=recs[qs][:])
                            nc.vector.tensor_add(
                                osb[:], osb[:],
                                bv_sb[:, eb * 512:(eb + 1) * 512])
                            row = qb * QB + qs * P
                            nc.sync.dma_start(
                                out_d[row:row + P, eb * 512:(eb + 1) * 512],
                                osb[:])
            if mode == "A":
                nc.gpsimd.dma_start(out_d[0:P, 0:8], yt[0][:, 0:8])
    nc.compile()
    return nc


def _get_nc(reps: int = 1, mode: str = "full"):
    key = (reps, mode)
    if key not in _NC_CACHE:
        _NC_CACHE[key] = build_nc(reps, mode)
    return _NC_CACHE[key]


def make_in_maps(x, Wq, bq, Wk, bk, Wv, bv):
    inv = np.float64(1.0 / np.sqrt(D))
    M2 = Wq.T.astype(np.float64) @ Wk.astype(np.float64) * inv
    z = Wk.T.astype(np.float64) @ bq.astype(np.float64) * inv
    m2b = np.ascontiguousarray(M2.astype(np.float32)).astype(BF16)
    wvT = np.ascontiguousarray(Wv.T).astype(BF16)
    z2 = np.ascontiguousarray(
        z.astype(np.float32).reshape(DT, P).T).astype(np.float32)
    bvb = np.ascontiguousarray(np.broadcast_to(bv, (P, D))).astype(np.float32)
    in_maps = []
    for c in range(N_CORES):
        b, h = divmod(c, 2)
        # rotate the sequence axis so this core's query half is always
        # columns/rows 0:QH -- attention is permutation-invariant along
        # the key axis as long as xT (pass1) and xN (pass2) agree.
        xr = np.roll(x[b], -h * QH, axis=0)
        xT = np.ascontiguousarray(xr.T).astype(BF16)
        xN = np.ascontiguousarray(xr).astype(BF16)
        in_maps.append({
            "xT": xT, "xN": xN,
            "M2": m2b, "WvT": wvT,
            "z2": z2, "bvb": bvb,
        })
    return in_maps


def kernel(x, Wq, bq, Wk, bk, Wv, bv):
    x = np.asarray(x, np.float32)
    in_maps = make_in_maps(x, np.asarray(Wq, np.float32),
                           np.asarray(bq, np.float32),
                           np.asarray(Wk, np.float32),
                           np.asarray(bk, np.float32),
                           np.asarray(Wv, np.float32),
                           np.asarray(bv, np.float32))
    nc = _get_nc()
    res = run_bass_kernel_spmd(nc, in_maps, core_ids=list(range(N_CORES)))
    out = np.empty((B, S, D), np.float32)
    for c in range(N_CORES):
        b, h = divmod(c, 2)
        out[b, h * QH:(h + 1) * QH, :] = res.results[c]["out"]
    return out

